# revision 27
# baseline (speedup 1.0000x reference)
"""Trainium2 Bass kernel for nn_CAM (DANet channel-attention module).

Per batch element b (one per NeuronCore, 8 cores data-parallel over B=8):
    xf = x[b].reshape(C, H*W)                       # [512, 4096]
    E = xf @ xf.T                                   # [512, 512] (symmetric)
    att = softmax(max_j(E) - E, axis=-1)            # inverted softmax
    out = gamma * (att @ xf) + x[b]

Kernel math (identical in exact arithmetic to the reference):
    c[i]    = min_j E[i, j]         (= column min by symmetry)
    W[j, i] = exp(c[i] - E[j, i])   (= numerator of att[i, j]; exponent <= 0)
    S[i]    = sum_j W[j, i]
    out[i]  = gamma * (1/S[i]) * sum_j W[j, i] * xf[j, :] + x[b][i, :]

Layout strategy:
  - xf natural  [c_part, n_free]  : [128, 4, 4096] f32 (residual + mm2 rhs via
                                    a free f32->f32r bitcast; f32r is bit-
                                    identical to f32, it only switches the PE
                                    pumping mode)
  - xf^T        [n_part, c_free]  : f32r via PE transposes, streamed per
                                    128-col k-slice through PSUM staging bufs
  - E           [i_part, j_free]  : 4 PSUM banks, fp32r matmuls over 32 k-tiles
  - W           [j_part, i_free]  : f32r; serves directly as lhsT of matmul2
                                    (no attention transpose needed, E symmetry)
  - c broadcast to the free axis stays on-chip: per-block PE transpose of the
    row-min vector + contraction-1 ones-matmul broadcast (no DRAM roundtrip)
  - W / S / 1/S are produced i-block-first so the first phase-2 matmul starts
    as soon as one i-block of W exists
  - phase-1 PSUM staging alternates between two pools whenever phase 2 is
    not using the second one, giving 4-deep staging in pure-phase-1 regions

reps > 1 unrolls the computation inside one NEFF with a true per-slab data
chain through DRAM scratch (rep k's stored slab feeds rep k+1's load of the
same columns).  Emission is software-pipelined: phase 2 of rep k-1 is
interleaved slab-by-slab with phase 1 of rep k (2-slab skew to cover the
store->load DMA latency), so the PE never drains between reps -- this measures
the true steady-state throughput of the kernel.
"""

import numpy as np

import concourse.bass as bass
import concourse.mybir as mybir
import concourse.tile as tile
from concourse import bacc
from concourse.masks import make_identity

P = 128          # partitions
C = 512          # channels
HW = 4096        # spatial (64*64)
CB = C // P      # 4 channel blocks
KB = HW // P     # 32 spatial blocks
NW = 512         # slab width (matmul free-dim chunk, DMA slab)
NCH = HW // NW   # 8 slabs
KPS = NW // P    # 4 k-slices per slab

F32 = mybir.dt.float32
F32R = mybir.dt.float32r
EXP = mybir.ActivationFunctionType.Exp
ALU = mybir.AluOpType
AX = mybir.AxisListType

# rep-0 input load schedule (col widths): small first loads so the PE
# pipeline starts early; must tile the 8 512-wide slabs exactly
LOADS = (256, 256, 512, 512, 512, 512, 512, 512, 512)
assert sum(LOADS) == HW

SKEW = 2         # phase-1 slab lag behind phase-2/store slab in the pipeline


def build_nc(reps: int = 1):
    nc = bacc.Bacc("TRN2", target_bir_lowering=False)
    x = nc.dram_tensor("x", [C, HW], F32, kind="ExternalInput")
    g = nc.dram_tensor("gamma", [1], F32, kind="ExternalInput")
    y = nc.dram_tensor("y", [C, HW], F32, kind="ExternalOutput")

    with tile.TileContext(nc) as tc:
        with (
            tc.tile_pool(name="consts", bufs=1) as consts,
            tc.tile_pool(name="xin", bufs=2) as xin_pool,
            tc.tile_pool(name="xtr", bufs=6) as xtr_pool,
            tc.tile_pool(name="w", bufs=2) as w_pool,
            tc.tile_pool(name="small", bufs=2) as small,
            tc.tile_pool(name="outp", bufs=3) as outp,
            tc.tile_pool(name="dram", bufs=1, space="DRAM") as dramp,
            tc.tile_pool(name="pxt", bufs=2, space="PSUM") as pxt_pool,
            tc.tile_pool(name="acc", bufs=4, space="PSUM") as acc_pool,
            tc.tile_pool(name="pop", bufs=2, space="PSUM") as po_pool,
        ):
            # constants (hoisted out of the rep loop)
            ident_f = consts.tile([P, P], F32)
            make_identity(nc, ident_f)
            ident = consts.tile([P, P], F32R)
            nc.scalar.copy(out=ident, in_=ident_f)
            ones_f = consts.tile([P, 8], F32)
            nc.vector.memset(ones_f, 1.0)
            ones = consts.tile([P, 8], F32R)
            nc.scalar.copy(out=ones, in_=ones_f)
            ones1 = consts.tile([1, P], F32)
            nc.vector.memset(ones1, 1.0)
            gamma_bc = consts.tile([P, 1], F32)
            nc.gpsimd.dma_start(out=gamma_bc, in_=g[:].partition_broadcast(P))

            xr = x.rearrange("(t p) n -> p t n", p=P)
            yr = y.rearrange("(t p) n -> p t n", p=P)

            # Per-slab DRAM scratch for reps > 1 (timing variants)
            if reps > 1:
                ybufs = [dramp.tile([C, NW], F32, tag=f"yb{s}", name=f"yb{s}")
                         for s in range(NCH)]
                ybrs = [b.rearrange("(t p) n -> p t n", p=P) for b in ybufs]

            # by symmetry only the upper block-triangle of E is computed by
            # matmuls; rhs column start per j-block (block (3,2) is recomputed
            # directly so every matmul keeps free dim >= 256)
            RS = (0, P, 2 * P, 2 * P)

            st = {}   # per-rep live tiles

            def alloc_rep(r):
                # X is loaded via casting SWDGE DMAs, so it is already
                # rounded to f32r for the PE; the residual add reads it
                # through a plain-f32 bitcast (identical bytes)
                X = xin_pool.tile([P, CB, HW], F32R, tag="x", name=f"X{r}")
                st[r] = dict(
                    X=X.bitcast(F32),
                    Xr=X,
                    W=w_pool.tile([P, CB, C], F32R, tag="w", name=f"W{r}"),
                    Wt=w_pool.tile([P, CB, C], F32, tag="wt", name=f"Wt{r}",
                                   bufs=1),
                    rowmin=small.tile([P, CB], F32, tag="rowmin", name=f"rm{r}",
                                      bufs=1),
                    c_bc=small.tile([P, NW], F32, tag="cbc", name=f"cbc{r}",
                                    bufs=1),
                    ct_sb=small.tile([1, CB, P], F32, tag="ctsb", name=f"ct{r}",
                                     bufs=1),
                    invsg=small.tile([P, CB], F32, tag="invsg", name=f"iv{r}"),
                    pe=[acc_pool.tile([P, C], F32, tag="acc", name=f"pe{r}_{j}")
                        for j in range(CB)],
                )

            def load(r, c0, c1):
                if r == 0:
                    src_ap = xr[:, :, c0:c1]
                else:
                    # rotated chain: slab s consumes rep r-1's output slab
                    # (s+1) % NCH -- still a full serializing data chain, but
                    # the last-processed slab depends on the first-stored one
                    s0 = (c0 // NW) * NW
                    sb = (c0 // NW + 1) % NCH
                    src_ap = ybrs[sb][:, :, c0 - s0:c1 - s0]
                nc.gpsimd.dma_start(out=st[r]["Xr"][:, :, c0:c1], in_=src_ap)

            def phase1_unit(r, k, alt_pool=False):
                d = st[r]
                pool = po_pool if alt_pool else pxt_pool
                tag = "po" if alt_pool else "pxt"
                pxt = pool.tile([P, C], F32R, tag=tag, name=f"pxt{r}_{k}")
                for t in range(CB):
                    nc.tensor.transpose(
                        pxt[:, t * P:(t + 1) * P],
                        d["Xr"][:, t, k * P:(k + 1) * P],
                        ident,
                    )
                xt2 = xtr_pool.tile([P, C], F32R, tag="xtk", name=f"xt{r}_{k}")
                if k % 2 == 0:
                    nc.vector.tensor_copy(out=xt2, in_=pxt.bitcast(F32))
                else:
                    nc.scalar.copy(out=xt2, in_=pxt.bitcast(F32))
                for jb in range(CB):
                    nc.tensor.matmul(
                        d["pe"][jb][:, RS[jb]:],
                        lhsT=xt2[:, jb * P:(jb + 1) * P],
                        rhs=xt2[:, RS[jb]:],
                        start=(k == 0),
                        stop=(k == KB - 1),
                    )

            def boundary(r):
                d = st[r]
                pe, rowmin = d["pe"], d["rowmin"]
                blk = small.tile([P, 5, P], F32, tag="blk", name=f"blk{r}",
                                 bufs=1)
                ct_ps = pxt_pool.tile([1, CB, P], F32, tag="pxt", name=f"ctp{r}")
                cb_ps = pxt_pool.tile([P, NW], F32, tag="pxt", name=f"cbp{r}")

                def c_block(t):
                    # rowmin column t -> [1, P] -> broadcast into c_bc cols
                    nc.tensor.transpose(
                        ct_ps[:, t, :], rowmin[:, t:t + 1], ident_f
                    )
                    if t % 2 == 0:
                        nc.vector.tensor_copy(
                            out=d["ct_sb"][:, t, :], in_=ct_ps[:, t, :]
                        )
                    else:
                        nc.scalar.copy(
                            out=d["ct_sb"][:, t, :], in_=ct_ps[:, t, :]
                        )
                    nc.tensor.matmul(
                        cb_ps[:, t * P:(t + 1) * P],
                        lhsT=ones1,
                        rhs=d["ct_sb"][:, t, :],
                        start=True,
                        stop=True,
                    )

                nc.vector.tensor_reduce(
                    out=rowmin[:, 0:1], in_=pe[0], axis=AX.X, op=ALU.min,
                )
                c_block(0)
                done = 0
                for bi in (1, 2, 3):
                    for bj in range(bi if bi < 3 else 2):
                        if done % 2 == 0:
                            nc.vector.tensor_copy(
                                out=blk[:, done, :],
                                in_=pe[bj][:, bi * P:(bi + 1) * P],
                            )
                        else:
                            nc.scalar.copy(
                                out=blk[:, done, :],
                                in_=pe[bj][:, bi * P:(bi + 1) * P],
                            )
                        nc.tensor.transpose(
                            pe[bi][:, bj * P:(bj + 1) * P],
                            blk[:, done, :], ident_f,
                        )
                        done += 1
                    nc.vector.tensor_reduce(
                        out=rowmin[:, bi:bi + 1], in_=pe[bi],
                        axis=AX.X, op=ALU.min,
                    )
                    c_block(bi)
                h = NW // 2
                nc.vector.tensor_copy(out=d["c_bc"][:, :h], in_=cb_ps[:, :h])
                nc.scalar.copy(out=d["c_bc"][:, h:], in_=cb_ps[:, h:])
                # W = exp(c - E)
                for jb in range(CB):
                    nc.vector.tensor_tensor(
                        out=d["Wt"][:, jb, :], in0=d["c_bc"], in1=pe[jb],
                        op=ALU.subtract,
                    )
                    nc.scalar.activation(
                        out=d["W"][:, jb, :], in_=d["Wt"][:, jb, :], func=EXP
                    )
                # S[i] and gamma/S[i] for all i-blocks
                ps_t = acc_pool.tile([P, CB, 8], F32, tag="acc", name=f"ps{r}")
                for ib in range(CB):
                    isl = slice(ib * P, (ib + 1) * P)
                    for jb in range(CB):
                        nc.tensor.matmul(
                            ps_t[:, ib, :],
                            lhsT=d["W"][:, jb, isl],
                            rhs=ones,
                            start=(jb == 0),
                            stop=(jb == CB - 1),
                        )
                for ib in range(CB):
                    nc.vector.reciprocal(
                        out=d["invsg"][:, ib:ib + 1], in_=ps_t[:, ib, 0:1]
                    )
                nc.vector.tensor_scalar_mul(d["invsg"], d["invsg"], gamma_bc)

            def phase2_slab(r, s):
                d = st[r]
                nsl = slice(s * NW, (s + 1) * NW)
                out_sb = outp.tile([P, CB, NW], F32, tag="osb", name=f"o{r}_{s}")
                for ib in range(CB):
                    isl = slice(ib * P, (ib + 1) * P)
                    po_t = po_pool.tile([P, NW], F32, tag="po",
                                        name=f"po{r}_{s}_{ib}")
                    for jb in range(CB):
                        nc.tensor.matmul(
                            po_t,
                            lhsT=d["W"][:, jb, isl],
                            rhs=d["Xr"][:, jb, nsl],
                            start=(jb == 0),
                            stop=(jb == CB - 1),
                        )
                    nc.vector.scalar_tensor_tensor(
                        out=out_sb[:, ib, :],
                        in0=po_t,
                        scalar=d["invsg"][:, ib:ib + 1],
                        in1=d["X"][:, ib, nsl],
                        op0=ALU.mult,
                        op1=ALU.add,
                    )
                if r == reps - 1:
                    tgt, csl = yr, nsl
                else:
                    tgt, csl = ybrs[s], slice(0, NW)
                if s < NCH - 1:
                    nc.scalar.dma_start(out=tgt[:, :, csl], in_=out_sb)
                else:
                    # last slab: store per i-block on alternating HWDGE rings
                    # so the final transfer is a single short one
                    for ib in range(CB):
                        eng = nc.scalar if ib % 2 == 0 else nc.sync
                        eng.dma_start(out=tgt[:, ib, csl], in_=out_sb[:, ib, :])

            # ---------------- program ----------------
            alloc_rep(0)
            col = 0
            for w_ld in LOADS:
                load(0, col, col + w_ld)
                col += w_ld
            for k in range(KB):
                phase1_unit(0, k, alt_pool=(k % 2 == 1))
            boundary(0)

            for r in range(1, reps):
                alloc_rep(r)
                for s in range(NCH + SKEW):
                    if s < NCH:
                        phase2_slab(r - 1, s)
                        if s >= 5:
                            load(r, s * NW, s * NW + NW // 2)
                            load(r, s * NW + NW // 2, (s + 1) * NW)
                        else:
                            load(r, s * NW, (s + 1) * NW)
                    if s >= SKEW:
                        # after the last phase-2 slab the po pool is idle:
                        # borrow it for 4-deep phase-1 staging
                        pure = s >= NCH
                        for ks in range(KPS):
                            k = (s - SKEW) * KPS + ks
                            phase1_unit(r, k, alt_pool=(pure and k % 2 == 1))
                boundary(r)
                del st[r - 1]

            for s in range(NCH):
                phase2_slab(reps - 1, s)

    nc.compile()
    return nc


_NC_CACHE = None


def _get_nc():
    global _NC_CACHE
    if _NC_CACHE is None:
        _NC_CACHE = build_nc()
    return _NC_CACHE


def kernel(x, gamma):
    from concourse.bass_utils import run_bass_kernel_spmd

    x = np.ascontiguousarray(np.asarray(x, dtype=np.float32))
    B = x.shape[0]
    assert x.shape == (8, C, 64, 64), x.shape
    xf = x.reshape(B, C, HW)
    gamma = np.ascontiguousarray(np.asarray(gamma, dtype=np.float32)).reshape(1)

    nc = _get_nc()
    in_maps = [{"x": xf[b], "gamma": gamma} for b in range(B)]
    res = run_bass_kernel_spmd(nc, in_maps, core_ids=list(range(B)))
    out = np.stack([res.results[b]["y"] for b in range(B)], axis=0)
    return out.reshape(B, C, 64, 64).astype(np.float32)


# revision 29
# speedup vs baseline: 1.0909x; 1.0909x over previous
"""Trainium2 Bass kernel for nn_CAM (DANet channel-attention module).

Per batch element b (one per NeuronCore, 8 cores data-parallel over B=8):
    xf = x[b].reshape(C, H*W)                       # [512, 4096]
    E = xf @ xf.T                                   # [512, 512] (symmetric)
    att = softmax(max_j(E) - E, axis=-1)            # inverted softmax
    out = gamma * (att @ xf) + x[b]

Kernel math (identical in exact arithmetic to the reference):
    c[i]    = min_j E[i, j]         (= column min by symmetry)
    W[j, i] = exp(c[i] - E[j, i])   (= numerator of att[i, j]; exponent <= 0)
    S[i]    = sum_j W[j, i]
    out[i]  = gamma * (1/S[i]) * sum_j W[j, i] * xf[j, :] + x[b][i, :]

Layout strategy:
  - xf natural  [c_part, n_free]  : [128, 4, 4096] f32 (residual + mm2 rhs via
                                    a free f32->f32r bitcast; f32r is bit-
                                    identical to f32, it only switches the PE
                                    pumping mode)
  - xf^T        [n_part, c_free]  : f32r via PE transposes, streamed per
                                    128-col k-slice through PSUM staging bufs
  - E           [i_part, j_free]  : 4 PSUM banks, fp32r matmuls over 32 k-tiles
  - W           [j_part, i_free]  : f32r; serves directly as lhsT of matmul2
                                    (no attention transpose needed, E symmetry)
  - c broadcast to the free axis stays on-chip: per-block PE transpose of the
    row-min vector + contraction-1 ones-matmul broadcast (no DRAM roundtrip)
  - W / S / 1/S are produced i-block-first so the first phase-2 matmul starts
    as soon as one i-block of W exists
  - phase-1 PSUM staging alternates between two pools whenever phase 2 is
    not using the second one, giving 4-deep staging in pure-phase-1 regions

reps > 1 unrolls the computation inside one NEFF with a true per-slab data
chain through DRAM scratch (rep k's stored slab feeds rep k+1's load of the
same columns).  Emission is software-pipelined: phase 2 of rep k-1 is
interleaved slab-by-slab with phase 1 of rep k (2-slab skew to cover the
store->load DMA latency), so the PE never drains between reps -- this measures
the true steady-state throughput of the kernel.
"""

import numpy as np

import concourse.bass as bass
import concourse.mybir as mybir
import concourse.tile as tile
from concourse import bacc
from concourse.masks import make_identity

P = 128          # partitions
C = 512          # channels
HW = 4096        # spatial (64*64)
CB = C // P      # 4 channel blocks
KB = HW // P     # 32 spatial blocks
NW = 512         # slab width (matmul free-dim chunk, DMA slab)
NCH = HW // NW   # 8 slabs
KPS = NW // P    # 4 k-slices per slab

F32 = mybir.dt.float32
F32R = mybir.dt.float32r
EXP = mybir.ActivationFunctionType.Exp
ALU = mybir.AluOpType
AX = mybir.AxisListType

# rep-0 input load schedule (col widths): small first loads so the PE
# pipeline starts early; must tile the 8 512-wide slabs exactly
LOADS = (256, 256, 512, 512, 512, 512, 512, 512, 512)
assert sum(LOADS) == HW

SKEW = 2         # phase-1 slab lag behind phase-2/store slab in the pipeline


def build_nc(reps: int = 1):
    nc = bacc.Bacc("TRN2", target_bir_lowering=False)
    x = nc.dram_tensor("x", [C, HW], F32, kind="ExternalInput")
    g = nc.dram_tensor("gamma", [1], F32, kind="ExternalInput")
    y = nc.dram_tensor("y", [C, HW], F32, kind="ExternalOutput")

    with tile.TileContext(nc) as tc:
        with (
            tc.tile_pool(name="consts", bufs=1) as consts,
            tc.tile_pool(name="xin", bufs=2) as xin_pool,
            tc.tile_pool(name="xtr", bufs=6) as xtr_pool,
            tc.tile_pool(name="w", bufs=2) as w_pool,
            tc.tile_pool(name="small", bufs=2) as small,
            tc.tile_pool(name="outp", bufs=3) as outp,
            tc.tile_pool(name="dram", bufs=1, space="DRAM") as dramp,
            tc.tile_pool(name="pxt", bufs=2, space="PSUM") as pxt_pool,
            tc.tile_pool(name="acc", bufs=4, space="PSUM") as acc_pool,
            tc.tile_pool(name="pop", bufs=2, space="PSUM") as po_pool,
        ):
            # constants (hoisted out of the rep loop)
            ident_f = consts.tile([P, P], F32)
            make_identity(nc, ident_f)
            ident = consts.tile([P, P], F32R)
            nc.scalar.copy(out=ident, in_=ident_f)
            ones_f = consts.tile([P, 8], F32)
            nc.vector.memset(ones_f, 1.0)
            ones = consts.tile([P, 8], F32R)
            nc.scalar.copy(out=ones, in_=ones_f)
            ones1 = consts.tile([1, P], F32)
            nc.vector.memset(ones1, 1.0)
            gamma_bc = consts.tile([P, 1], F32)
            nc.gpsimd.dma_start(out=gamma_bc, in_=g[:].partition_broadcast(P))

            xr = x.rearrange("(t p) n -> p t n", p=P)
            yr = y.rearrange("(t p) n -> p t n", p=P)

            # Per-slab DRAM scratch for reps > 1 (timing variants)
            if reps > 1:
                ybufs = [dramp.tile([C, NW], F32, tag=f"yb{s}", name=f"yb{s}")
                         for s in range(NCH)]
                ybrs = [b.rearrange("(t p) n -> p t n", p=P) for b in ybufs]

            # by symmetry only the upper block-triangle of E is computed by
            # matmuls; rhs column start per j-block (block (3,2) is recomputed
            # directly so every matmul keeps free dim >= 256)
            RS = (0, P, 2 * P, 2 * P)

            st = {}   # per-rep live tiles

            def alloc_rep(r):
                # X is loaded via casting SWDGE DMAs, so it is already
                # rounded to f32r for the PE; the residual add reads it
                # through a plain-f32 bitcast (identical bytes)
                X = xin_pool.tile([P, CB, HW], F32R, tag="x", name=f"X{r}")
                st[r] = dict(
                    X=X.bitcast(F32),
                    Xr=X,
                    W=w_pool.tile([P, CB, C], F32R, tag="w", name=f"W{r}"),
                    Wt=w_pool.tile([P, CB, C], F32, tag="wt", name=f"Wt{r}",
                                   bufs=1),
                    rowmin=small.tile([P, CB], F32, tag="rowmin", name=f"rm{r}",
                                      bufs=1),
                    c_bc=small.tile([P, NW], F32, tag="cbc", name=f"cbc{r}",
                                    bufs=1),
                    ct_sb=small.tile([1, CB, P], F32, tag="ctsb", name=f"ct{r}",
                                     bufs=1),
                    invsg=small.tile([P, CB], F32, tag="invsg", name=f"iv{r}"),
                    pe=[acc_pool.tile([P, C], F32, tag="acc", name=f"pe{r}_{j}")
                        for j in range(CB)],
                )

            def load(r, c0, c1):
                if r == 0:
                    src_ap = xr[:, :, c0:c1]
                else:
                    # rotated chain: slab s consumes rep r-1's output slab
                    # (s+1) % NCH -- still a full serializing data chain, but
                    # the last-processed slab depends on the first-stored one
                    s0 = (c0 // NW) * NW
                    sb = (c0 // NW + 1) % NCH
                    src_ap = ybrs[sb][:, :, c0 - s0:c1 - s0]
                nc.gpsimd.dma_start(out=st[r]["Xr"][:, :, c0:c1], in_=src_ap)

            def emit_s_invsg(r):
                d = st[r]
                ps_t = acc_pool.tile([P, CB, 8], F32, tag="acc", name=f"ps{r}")
                for ib in range(CB):
                    isl = slice(ib * P, (ib + 1) * P)
                    for jb in range(CB):
                        nc.tensor.matmul(
                            ps_t[:, ib, :],
                            lhsT=d["W"][:, jb, isl],
                            rhs=ones,
                            start=(jb == 0),
                            stop=(jb == CB - 1),
                        )
                for ib in range(CB):
                    nc.vector.reciprocal(
                        out=d["invsg"][:, ib:ib + 1], in_=ps_t[:, ib, 0:1]
                    )
                nc.vector.tensor_scalar_mul(d["invsg"], d["invsg"], gamma_bc)

            def phase1_unit(r, k, alt_pool=False):
                d = st[r]
                pool = po_pool if alt_pool else pxt_pool
                tag = "po" if alt_pool else "pxt"
                pxt = pool.tile([P, C], F32R, tag=tag, name=f"pxt{r}_{k}")
                for t in range(CB):
                    nc.tensor.transpose(
                        pxt[:, t * P:(t + 1) * P],
                        d["Xr"][:, t, k * P:(k + 1) * P],
                        ident,
                    )
                xt2 = xtr_pool.tile([P, C], F32R, tag="xtk", name=f"xt{r}_{k}")
                if k % 2 == 0:
                    nc.vector.tensor_copy(out=xt2, in_=pxt.bitcast(F32))
                else:
                    nc.scalar.copy(out=xt2, in_=pxt.bitcast(F32))
                for jb in range(CB):
                    nc.tensor.matmul(
                        d["pe"][jb][:, RS[jb]:],
                        lhsT=xt2[:, jb * P:(jb + 1) * P],
                        rhs=xt2[:, RS[jb]:],
                        start=(k == 0),
                        stop=(k == KB - 1),
                    )

            def boundary(r):
                d = st[r]
                pe, rowmin = d["pe"], d["rowmin"]
                blk = small.tile([P, 5, P], F32, tag="blk", name=f"blk{r}",
                                 bufs=1)
                ct_ps = pxt_pool.tile([1, CB, P], F32, tag="pxt", name=f"ctp{r}")
                cb_ps = pxt_pool.tile([P, NW], F32, tag="pxt", name=f"cbp{r}")

                def c_block(t):
                    # rowmin column t -> [1, P] -> broadcast into c_bc cols
                    nc.tensor.transpose(
                        ct_ps[:, t, :], rowmin[:, t:t + 1], ident_f
                    )
                    if t % 2 == 0:
                        nc.vector.tensor_copy(
                            out=d["ct_sb"][:, t, :], in_=ct_ps[:, t, :]
                        )
                    else:
                        nc.scalar.copy(
                            out=d["ct_sb"][:, t, :], in_=ct_ps[:, t, :]
                        )
                    nc.tensor.matmul(
                        cb_ps[:, t * P:(t + 1) * P],
                        lhsT=ones1,
                        rhs=d["ct_sb"][:, t, :],
                        start=True,
                        stop=True,
                    )

                nc.vector.tensor_reduce(
                    out=rowmin[:, 0:1], in_=pe[0], axis=AX.X, op=ALU.min,
                )
                c_block(0)
                done = 0
                for bi in (1, 2, 3):
                    for bj in range(bi if bi < 3 else 2):
                        if done % 2 == 0:
                            nc.vector.tensor_copy(
                                out=blk[:, done, :],
                                in_=pe[bj][:, bi * P:(bi + 1) * P],
                            )
                        else:
                            nc.scalar.copy(
                                out=blk[:, done, :],
                                in_=pe[bj][:, bi * P:(bi + 1) * P],
                            )
                        nc.tensor.transpose(
                            pe[bi][:, bj * P:(bj + 1) * P],
                            blk[:, done, :], ident_f,
                        )
                        done += 1
                    nc.vector.tensor_reduce(
                        out=rowmin[:, bi:bi + 1], in_=pe[bi],
                        axis=AX.X, op=ALU.min,
                    )
                    c_block(bi)
                h = NW // 2
                nc.vector.tensor_copy(out=d["c_bc"][:, :h], in_=cb_ps[:, :h])
                nc.scalar.copy(out=d["c_bc"][:, h:], in_=cb_ps[:, h:])
                # W = exp(c - E)
                for jb in range(CB):
                    nc.vector.tensor_tensor(
                        out=d["Wt"][:, jb, :], in0=d["c_bc"], in1=pe[jb],
                        op=ALU.subtract,
                    )
                    nc.scalar.activation(
                        out=d["W"][:, jb, :], in_=d["Wt"][:, jb, :], func=EXP
                    )
                emit_s_invsg(r)

            def phase2_slab(r, s):
                d = st[r]
                nsl = slice(s * NW, (s + 1) * NW)
                out_sb = outp.tile([P, CB, NW], F32, tag="osb", name=f"o{r}_{s}")
                for ib in range(CB):
                    isl = slice(ib * P, (ib + 1) * P)
                    po_t = po_pool.tile([P, NW], F32, tag="po",
                                        name=f"po{r}_{s}_{ib}")
                    for jb in range(CB):
                        nc.tensor.matmul(
                            po_t,
                            lhsT=d["W"][:, jb, isl],
                            rhs=d["Xr"][:, jb, nsl],
                            start=(jb == 0),
                            stop=(jb == CB - 1),
                        )
                    nc.vector.scalar_tensor_tensor(
                        out=out_sb[:, ib, :],
                        in0=po_t,
                        scalar=d["invsg"][:, ib:ib + 1],
                        in1=d["X"][:, ib, nsl],
                        op0=ALU.mult,
                        op1=ALU.add,
                    )
                if r == reps - 1:
                    tgt, csl = yr, nsl
                else:
                    tgt, csl = ybrs[s], slice(0, NW)
                if s < NCH - 1:
                    nc.scalar.dma_start(out=tgt[:, :, csl], in_=out_sb)
                else:
                    # last slab: store per i-block on alternating HWDGE rings
                    # so the final transfer is a single short one
                    for ib in range(CB):
                        eng = nc.scalar if ib % 2 == 0 else nc.sync
                        eng.dma_start(out=tgt[:, ib, csl], in_=out_sb[:, ib, :])

            # ---------------- program ----------------
            alloc_rep(0)
            col = 0
            for w_ld in LOADS:
                load(0, col, col + w_ld)
                col += w_ld
            for k in range(KB):
                phase1_unit(0, k, alt_pool=(k % 2 == 1))
            boundary(0)

            for r in range(1, reps):
                alloc_rep(r)
                for s in range(NCH + SKEW):
                    if s < NCH:
                        phase2_slab(r - 1, s)
                        if s >= 5:
                            load(r, s * NW, s * NW + NW // 2)
                            load(r, s * NW + NW // 2, (s + 1) * NW)
                        else:
                            load(r, s * NW, (s + 1) * NW)
                    if s >= SKEW:
                        # after the last phase-2 slab the po pool is idle:
                        # borrow it for 4-deep phase-1 staging
                        pure = s >= NCH
                        for ks in range(KPS):
                            k = (s - SKEW) * KPS + ks
                            phase1_unit(r, k, alt_pool=(pure and k % 2 == 1))
                boundary(r)
                del st[r - 1]

            for s in range(NCH):
                phase2_slab(reps - 1, s)

    nc.compile()
    return nc


_NC_CACHE = None


def _get_nc():
    global _NC_CACHE
    if _NC_CACHE is None:
        _NC_CACHE = build_nc()
    return _NC_CACHE


def kernel(x, gamma):
    from concourse.bass_utils import run_bass_kernel_spmd

    x = np.ascontiguousarray(np.asarray(x, dtype=np.float32))
    B = x.shape[0]
    assert x.shape == (8, C, 64, 64), x.shape
    xf = x.reshape(B, C, HW)
    gamma = np.ascontiguousarray(np.asarray(gamma, dtype=np.float32)).reshape(1)

    nc = _get_nc()
    in_maps = [{"x": xf[b], "gamma": gamma} for b in range(B)]
    res = run_bass_kernel_spmd(nc, in_maps, core_ids=list(range(B)))
    out = np.stack([res.results[b]["y"] for b in range(B)], axis=0)
    return out.reshape(B, C, 64, 64).astype(np.float32)


# revision 30
# speedup vs baseline: 1.2547x; 1.1501x over previous
"""Trainium2 Bass kernel for nn_CAM (DANet channel-attention module).

Per batch element b (one per NeuronCore, 8 cores data-parallel over B=8):
    xf = x[b].reshape(C, H*W)                       # [512, 4096]
    E = xf @ xf.T                                   # [512, 512] (symmetric)
    att = softmax(max_j(E) - E, axis=-1)            # inverted softmax
    out = gamma * (att @ xf) + x[b]

Kernel math (identical in exact arithmetic to the reference):
    c[i]    = min_j E[i, j]         (= column min by symmetry)
    W[j, i] = exp(c[i] - E[j, i])   (= numerator of att[i, j]; exponent <= 0)
    S[i]    = sum_j W[j, i]
    out[i]  = gamma * (1/S[i]) * sum_j W[j, i] * xf[j, :] + x[b][i, :]

Layout strategy:
  - xf natural  [c_part, n_free]  : [128, 4, 4096] f32 (residual + mm2 rhs via
                                    a free f32->f32r bitcast; f32r is bit-
                                    identical to f32, it only switches the PE
                                    pumping mode)
  - xf^T        [n_part, c_free]  : f32r via PE transposes, streamed per
                                    128-col k-slice through PSUM staging bufs
  - E           [i_part, j_free]  : 4 PSUM banks, fp32r matmuls over 32 k-tiles
  - W           [j_part, i_free]  : f32r; serves directly as lhsT of matmul2
                                    (no attention transpose needed, E symmetry)
  - c broadcast to the free axis stays on-chip: per-block PE transpose of the
    row-min vector + contraction-1 ones-matmul broadcast (no DRAM roundtrip)
  - W / S / 1/S are produced i-block-first so the first phase-2 matmul starts
    as soon as one i-block of W exists
  - phase-1 PSUM staging alternates between two pools whenever phase 2 is
    not using the second one, giving 4-deep staging in pure-phase-1 regions

reps > 1 unrolls the computation inside one NEFF with a true per-slab data
chain through DRAM scratch (rep k's stored slab feeds rep k+1's load of the
same columns).  Emission is software-pipelined: phase 2 of rep k-1 is
interleaved slab-by-slab with phase 1 of rep k (2-slab skew to cover the
store->load DMA latency), so the PE never drains between reps -- this measures
the true steady-state throughput of the kernel.
"""

import numpy as np

import concourse.bass as bass
import concourse.mybir as mybir
import concourse.tile as tile
from concourse import bacc
from concourse.masks import make_identity

P = 128          # partitions
C = 512          # channels
HW = 4096        # spatial (64*64)
CB = C // P      # 4 channel blocks
KB = HW // P     # 32 spatial blocks
NW = 512         # slab width (matmul free-dim chunk, DMA slab)
NCH = HW // NW   # 8 slabs
KPS = NW // P    # 4 k-slices per slab

F32 = mybir.dt.float32
F32R = mybir.dt.float32r
EXP = mybir.ActivationFunctionType.Exp
ALU = mybir.AluOpType
AX = mybir.AxisListType

# rep-0 input load schedule (col widths): small first loads so the PE
# pipeline starts early; boundaries must stay within single 512-wide slabs
LOADS = (128, 128, 256, 512, 512, 512, 512, 512, 512, 512)
assert sum(LOADS) == HW

SKEW = 2         # phase-1 slab lag behind phase-2/store slab in the pipeline


def build_nc(reps: int = 1):
    nc = bacc.Bacc("TRN2", target_bir_lowering=False)
    x = nc.dram_tensor("x", [C, HW], F32, kind="ExternalInput")
    g = nc.dram_tensor("gamma", [1], F32, kind="ExternalInput")
    y = nc.dram_tensor("y", [C, HW], F32, kind="ExternalOutput")

    with tile.TileContext(nc) as tc:
        with (
            tc.tile_pool(name="consts", bufs=1) as consts,
            tc.tile_pool(name="xin", bufs=2) as xin_pool,
            tc.tile_pool(name="xtr", bufs=6) as xtr_pool,
            tc.tile_pool(name="w", bufs=2) as w_pool,
            tc.tile_pool(name="small", bufs=2) as small,
            tc.tile_pool(name="outp", bufs=3) as outp,
            tc.tile_pool(name="dram", bufs=1, space="DRAM") as dramp,
            tc.tile_pool(name="pxt", bufs=2, space="PSUM") as pxt_pool,
            tc.tile_pool(name="acc", bufs=4, space="PSUM") as acc_pool,
            tc.tile_pool(name="pop", bufs=2, space="PSUM") as po_pool,
        ):
            # constants (hoisted out of the rep loop)
            ident_f = consts.tile([P, P], F32)
            make_identity(nc, ident_f)
            ident = consts.tile([P, P], F32R)
            nc.scalar.copy(out=ident, in_=ident_f)
            ones_f = consts.tile([P, 8], F32)
            nc.vector.memset(ones_f, 1.0)
            ones = consts.tile([P, 8], F32R)
            nc.scalar.copy(out=ones, in_=ones_f)
            ones1 = consts.tile([1, P], F32)
            nc.vector.memset(ones1, 1.0)
            gamma_bc = consts.tile([P, 1], F32)
            nc.gpsimd.dma_start(out=gamma_bc, in_=g[:].partition_broadcast(P))

            xr = x.rearrange("(t p) n -> p t n", p=P)
            yr = y.rearrange("(t p) n -> p t n", p=P)

            # Per-slab DRAM scratch for reps > 1 (timing variants)
            if reps > 1:
                ybufs = [dramp.tile([C, NW], F32, tag=f"yb{s}", name=f"yb{s}")
                         for s in range(NCH)]
                ybrs = [b.rearrange("(t p) n -> p t n", p=P) for b in ybufs]

            # by symmetry only the upper block-triangle of E is computed by
            # matmuls; rhs column start per j-block (block (3,2) is recomputed
            # directly so every matmul keeps free dim >= 256)
            RS = (0, P, 2 * P, 2 * P)

            st = {}   # per-rep live tiles

            def alloc_rep(r):
                # X is loaded via casting SWDGE DMAs, so it is already
                # rounded to f32r for the PE; the residual add reads it
                # through a plain-f32 bitcast (identical bytes)
                X = xin_pool.tile([P, CB, HW], F32R, tag="x", name=f"X{r}")
                st[r] = dict(
                    X=X.bitcast(F32),
                    Xr=X,
                    W=w_pool.tile([P, CB, C], F32R, tag="w", name=f"W{r}"),
                    Wt=w_pool.tile([P, CB, C], F32, tag="wt", name=f"Wt{r}",
                                   bufs=1),
                    rowmin=small.tile([P, CB], F32, tag="rowmin", name=f"rm{r}",
                                      bufs=1),
                    c_bc=small.tile([P, NW], F32, tag="cbc", name=f"cbc{r}",
                                    bufs=1),
                    ct_sb=small.tile([1, CB, P], F32, tag="ctsb", name=f"ct{r}",
                                     bufs=1),
                    invsg=small.tile([P, CB], F32, tag="invsg", name=f"iv{r}"),
                    pe=[acc_pool.tile([P, C], F32, tag="acc", name=f"pe{r}_{j}")
                        for j in range(CB)],
                )

            def load(r, c0, c1):
                if r == 0:
                    src_ap = xr[:, :, c0:c1]
                else:
                    # rotated chain: slab s consumes rep r-1's output slab
                    # (s+1) % NCH -- still a full serializing data chain, but
                    # the last-processed slab depends on the first-stored one
                    s0 = (c0 // NW) * NW
                    sb = (c0 // NW + 1) % NCH
                    src_ap = ybrs[sb][:, :, c0 - s0:c1 - s0]
                nc.gpsimd.dma_start(out=st[r]["Xr"][:, :, c0:c1], in_=src_ap)

            def emit_s_invsg(r):
                d = st[r]
                ps_t = acc_pool.tile([P, CB, 8], F32, tag="acc", name=f"ps{r}")
                for ib in range(CB):
                    isl = slice(ib * P, (ib + 1) * P)
                    for jb in range(CB):
                        nc.tensor.matmul(
                            ps_t[:, ib, :],
                            lhsT=d["W"][:, jb, isl],
                            rhs=ones,
                            start=(jb == 0),
                            stop=(jb == CB - 1),
                        )
                for ib in range(CB):
                    nc.vector.reciprocal(
                        out=d["invsg"][:, ib:ib + 1], in_=ps_t[:, ib, 0:1]
                    )
                nc.vector.tensor_scalar_mul(d["invsg"], d["invsg"], gamma_bc)

            def phase1_unit(r, k, alt_pool=False):
                d = st[r]
                pool = po_pool if alt_pool else pxt_pool
                tag = "po" if alt_pool else "pxt"
                pxt = pool.tile([P, C], F32R, tag=tag, name=f"pxt{r}_{k}")
                for t in range(CB):
                    nc.tensor.transpose(
                        pxt[:, t * P:(t + 1) * P],
                        d["Xr"][:, t, k * P:(k + 1) * P],
                        ident,
                    )
                xt2 = xtr_pool.tile([P, C], F32R, tag="xtk", name=f"xt{r}_{k}")
                if k % 2 == 0:
                    nc.vector.tensor_copy(out=xt2, in_=pxt.bitcast(F32))
                else:
                    nc.scalar.copy(out=xt2, in_=pxt.bitcast(F32))
                for jb in range(CB):
                    nc.tensor.matmul(
                        d["pe"][jb][:, RS[jb]:],
                        lhsT=xt2[:, jb * P:(jb + 1) * P],
                        rhs=xt2[:, RS[jb]:],
                        start=(k == 0),
                        stop=(k == KB - 1),
                    )

            def boundary(r):
                d = st[r]
                pe, rowmin = d["pe"], d["rowmin"]
                blk = small.tile([P, 5, P], F32, tag="blk", name=f"blk{r}",
                                 bufs=1)
                ct_ps = pxt_pool.tile([1, CB, P], F32, tag="pxt", name=f"ctp{r}")
                cb_ps = pxt_pool.tile([P, NW], F32, tag="pxt", name=f"cbp{r}")

                def c_block(t):
                    # rowmin column t -> [1, P] -> broadcast into c_bc cols
                    nc.tensor.transpose(
                        ct_ps[:, t, :], rowmin[:, t:t + 1], ident_f
                    )
                    if t % 2 == 0:
                        nc.vector.tensor_copy(
                            out=d["ct_sb"][:, t, :], in_=ct_ps[:, t, :]
                        )
                    else:
                        nc.scalar.copy(
                            out=d["ct_sb"][:, t, :], in_=ct_ps[:, t, :]
                        )
                    nc.tensor.matmul(
                        cb_ps[:, t * P:(t + 1) * P],
                        lhsT=ones1,
                        rhs=d["ct_sb"][:, t, :],
                        start=True,
                        stop=True,
                    )

                nc.vector.tensor_reduce(
                    out=rowmin[:, 0:1], in_=pe[0], axis=AX.X, op=ALU.min,
                )
                c_block(0)
                done = 0
                for bi in (1, 2, 3):
                    for bj in range(bi if bi < 3 else 2):
                        if done % 2 == 0:
                            nc.vector.tensor_copy(
                                out=blk[:, done, :],
                                in_=pe[bj][:, bi * P:(bi + 1) * P],
                            )
                        else:
                            nc.scalar.copy(
                                out=blk[:, done, :],
                                in_=pe[bj][:, bi * P:(bi + 1) * P],
                            )
                        nc.tensor.transpose(
                            pe[bi][:, bj * P:(bj + 1) * P],
                            blk[:, done, :], ident_f,
                        )
                        done += 1
                    nc.vector.tensor_reduce(
                        out=rowmin[:, bi:bi + 1], in_=pe[bi],
                        axis=AX.X, op=ALU.min,
                    )
                    c_block(bi)
                h = NW // 2
                nc.vector.tensor_copy(out=d["c_bc"][:, :h], in_=cb_ps[:, :h])
                nc.scalar.copy(out=d["c_bc"][:, h:], in_=cb_ps[:, h:])
                # W = exp(c - E)
                for jb in range(CB):
                    nc.vector.tensor_tensor(
                        out=d["Wt"][:, jb, :], in0=d["c_bc"], in1=pe[jb],
                        op=ALU.subtract,
                    )
                    nc.scalar.activation(
                        out=d["W"][:, jb, :], in_=d["Wt"][:, jb, :], func=EXP
                    )
                emit_s_invsg(r)

            def phase2_slab(r, s):
                d = st[r]
                nsl = slice(s * NW, (s + 1) * NW)
                out_sb = outp.tile([P, CB, NW], F32, tag="osb", name=f"o{r}_{s}")
                for ib in range(CB):
                    isl = slice(ib * P, (ib + 1) * P)
                    po_t = po_pool.tile([P, NW], F32, tag="po",
                                        name=f"po{r}_{s}_{ib}")
                    for jb in range(CB):
                        nc.tensor.matmul(
                            po_t,
                            lhsT=d["W"][:, jb, isl],
                            rhs=d["Xr"][:, jb, nsl],
                            start=(jb == 0),
                            stop=(jb == CB - 1),
                        )
                    nc.vector.scalar_tensor_tensor(
                        out=out_sb[:, ib, :],
                        in0=po_t,
                        scalar=d["invsg"][:, ib:ib + 1],
                        in1=d["X"][:, ib, nsl],
                        op0=ALU.mult,
                        op1=ALU.add,
                    )
                if r == reps - 1:
                    tgt, csl = yr, nsl
                else:
                    tgt, csl = ybrs[s], slice(0, NW)
                if s < NCH - 1:
                    nc.scalar.dma_start(out=tgt[:, :, csl], in_=out_sb)
                else:
                    # last slab: store per i-block on alternating HWDGE rings
                    # so the final transfer is a single short one
                    for ib in range(CB):
                        eng = nc.scalar if ib % 2 == 0 else nc.sync
                        eng.dma_start(out=tgt[:, ib, csl], in_=out_sb[:, ib, :])

            # ---------------- program ----------------
            alloc_rep(0)
            col = 0
            for w_ld in LOADS:
                load(0, col, col + w_ld)
                col += w_ld
            for k in range(KB):
                phase1_unit(0, k, alt_pool=(k % 2 == 1))
            boundary(0)

            for r in range(1, reps):
                alloc_rep(r)
                for s in range(NCH + SKEW):
                    if s < NCH:
                        phase2_slab(r - 1, s)
                        if s >= 5:
                            load(r, s * NW, s * NW + NW // 2)
                            load(r, s * NW + NW // 2, (s + 1) * NW)
                        else:
                            load(r, s * NW, (s + 1) * NW)
                    if s >= SKEW:
                        # after the last phase-2 slab the po pool is idle:
                        # borrow it for 4-deep phase-1 staging
                        pure = s >= NCH
                        for ks in range(KPS):
                            k = (s - SKEW) * KPS + ks
                            phase1_unit(r, k, alt_pool=(pure and k % 2 == 1))
                boundary(r)
                del st[r - 1]

            for s in range(NCH):
                phase2_slab(reps - 1, s)

    nc.compile()
    return nc


_NC_CACHE = None


def _get_nc():
    global _NC_CACHE
    if _NC_CACHE is None:
        _NC_CACHE = build_nc()
    return _NC_CACHE


def kernel(x, gamma):
    from concourse.bass_utils import run_bass_kernel_spmd

    x = np.ascontiguousarray(np.asarray(x, dtype=np.float32))
    B = x.shape[0]
    assert x.shape == (8, C, 64, 64), x.shape
    xf = x.reshape(B, C, HW)
    gamma = np.ascontiguousarray(np.asarray(gamma, dtype=np.float32)).reshape(1)

    nc = _get_nc()
    in_maps = [{"x": xf[b], "gamma": gamma} for b in range(B)]
    res = run_bass_kernel_spmd(nc, in_maps, core_ids=list(range(B)))
    out = np.stack([res.results[b]["y"] for b in range(B)], axis=0)
    return out.reshape(B, C, 64, 64).astype(np.float32)
